# revision 4
# baseline (speedup 1.0000x reference)
"""GCN encoder (2-layer GCNConv) on 8 Trainium2 NeuronCores.

Strategy (graph/data parallel, edges partitioned by destination):
- Nodes sharded 12500/core. Layer tables (node features after the dense
  transform) live in each core's HBM at 256B row pitch, fp16.
- L1: every core recomputes h1 = x@W1 for ALL nodes (cheap, avoids a big
  collective); L2 exchanges the small transformed shard via AllGather.
- Message passing: per-edge rows fetched with the GPSIMD dma_gather ucode
  (4 SWDGE queues in parallel), aggregated per 128-dst block with one-hot
  matmuls (S = is_equal(iota, dst) * norm) accumulating in PSUM, flushed
  into an SBUF accumulator. Self-loops are ordinary edges.
"""
import os
import sys

sys.path.insert(0, '/opt/trn_rl_repo')

import numpy as np

import concourse.bass as bass
import concourse.bacc as bacc
import concourse.mybir as mybir
import concourse.tile as tile
from concourse import bass_utils
from concourse.bass import exact_div
from concourse.tile_rust import add_dep_helper

P = 128
N = 100000
NCORE = 8
NPC = N // NCORE            # 12500 nodes per core
NBLK = (NPC + P - 1) // P   # 98 blocks per core
RPC = NBLK * P              # 12544 table rows per core shard
TROWS = NCORE * RPC         # 100352
WIN = 32768                 # int16-addressable window
NWIN = (TROWS + WIN - 1) // WIN  # 4
IN_C = 128
HID = 64
OUT_C = 32
NI = 896                    # idxs per gather call (7 chunks; 57 ring descs)
CHUNKS_PER_CALL = NI // P   # 7
N16 = NI // 16              # 56

last_exec_time_ns = None
_cache = {}


# ---------------------------------------------------------------- host prep

def _table_row(n):
    """node id -> table row (p-major within each core shard)."""
    m = n // NPC
    l = n - m * NPC
    return m * RPC + (l % P) * NBLK + l // P


def _wrap_idx_batch(calls_idx):
    """[ncalls, NI] int16 -> [128, ncalls*N16] wrapped (16-part, replicated 8x)."""
    ncalls = calls_idx.shape[0]
    w = calls_idx.reshape(ncalls, N16, 16).transpose(0, 2, 1)  # [ncalls, 16, N16]
    w = np.tile(w, (1, 8, 1))                                   # [ncalls, 128, N16]
    return np.ascontiguousarray(w.transpose(1, 0, 2).reshape(P, ncalls * N16))


def _prep_layer(src_rows, dst, norm):
    """Build the common program structure + per-core padded edge data.

    src_rows: table row of each edge's source (int64)
    dst: destination node id (int64), norm: fp32
    Returns (meta, per_core_data).
    """
    m = dst // NPC
    l = dst - m * NPC
    blk = l // P
    dib = (l % P).astype(np.float32)
    w = src_rows // WIN
    widx = (src_rows - w * WIN).astype(np.int16)

    group = (m * NWIN + w) * NBLK + blk          # (core, window, block)
    counts = np.bincount(group, minlength=NCORE * NWIN * NBLK)
    counts = counts.reshape(NCORE, NWIN, NBLK)
    # common structure: max chunks over cores per (window, block)
    gchunks = np.maximum(1, -(-counts.max(axis=0) // P))   # [NWIN, NBLK] >=1
    # per-window chunk count padded to whole calls
    wchunks_raw = gchunks.sum(axis=1)                      # [NWIN]
    wcalls = -(-wchunks_raw // CHUNKS_PER_CALL)
    wchunks = wcalls * CHUNKS_PER_CALL
    total_chunks = int(wchunks.sum())
    total_calls = int(wcalls.sum())

    # group -> chunk offsets (window-local), groups ordered by block within window
    gchunk_off = np.zeros((NWIN, NBLK), np.int64)
    for wi in range(NWIN):
        gchunk_off[wi] = np.cumsum(gchunks[wi]) - gchunks[wi]
    chunk_base = np.cumsum(wchunks) - wchunks              # global chunk base per window
    call_base = np.cumsum(wcalls) - wcalls

    # place each edge into the padded layout (per core)
    order = np.lexsort((blk, w, m))
    m_s, w_s, blk_s, widx_s, dib_s, norm_s = (a[order] for a in (m, w, blk, widx, dib, norm))
    g_s = (m_s * NWIN + w_s) * NBLK + blk_s
    # position within its group
    gstart = np.zeros(NCORE * NWIN * NBLK + 1, np.int64)
    np.cumsum(np.bincount(g_s, minlength=NCORE * NWIN * NBLK), out=gstart[1:])
    within = np.arange(len(g_s)) - gstart[g_s]
    # padded slot (window-local edge index)
    slot = (gchunk_off[w_s, blk_s] * P + within).astype(np.int64)

    per_core = []
    for mi in range(NCORE):
        sel = m_s == mi
        idx_arr = np.zeros((NWIN, int(wchunks.max()) * P), np.int16)  # pad idx = 0 (valid row)
        dst_arr = np.zeros(total_chunks * P, np.float32)
        norm_arr = np.zeros(total_chunks * P, np.float32)
        sw, sslot = w_s[sel], slot[sel]
        idx_arr[sw, sslot] = widx_s[sel]
        gidx = (chunk_base[sw] * P + sslot)
        dst_arr[gidx] = dib_s[sel]
        norm_arr[gidx] = norm_s[sel]
        # idx calls: concat per window
        calls_idx = np.concatenate(
            [idx_arr[wi, : wchunks[wi] * P].reshape(-1, NI) for wi in range(NWIN)], axis=0)
        wrapped = _wrap_idx_batch(calls_idx)
        # [p, chunk] layout for dst/norm: edge i -> (p=i%128, chunk=i//128)
        dstm = np.ascontiguousarray(dst_arr.reshape(total_chunks, P).T)
        normm = np.ascontiguousarray(norm_arr.reshape(total_chunks, P).T)
        per_core.append((wrapped, dstm, normm))

    meta = {
        "wcalls": wcalls.tolist(),
        "wchunks": wchunks.tolist(),
        "gchunks": gchunks,
        "gchunk_off": gchunk_off,
        "chunk_base": chunk_base.tolist(),
        "call_base": call_base.tolist(),
        "total_chunks": total_chunks,
        "total_calls": total_calls,
    }
    return meta, per_core


def _host_prep(x, edge_index, W1, b1, W2, b2):
    src = edge_index[0].astype(np.int64)
    dst = edge_index[1].astype(np.int64)
    deg = np.bincount(dst, minlength=N).astype(np.float64) + 1.0
    dinv = 1.0 / np.sqrt(deg)
    # append self-loops as ordinary edges
    loops = np.arange(N, dtype=np.int64)
    src_a = np.concatenate([src, loops])
    dst_a = np.concatenate([dst, loops])
    norm_a = (dinv[src_a] * dinv[dst_a]).astype(np.float32)
    src_rows = _table_row(src_a)

    meta1, pc1 = _prep_layer(src_rows, dst_a, norm_a)
    meta2, pc2 = meta1, pc1  # same edges both layers

    # xT fp16 [128, TROWS] block-major columns: c = m*RPC + b*128 + p -> node m*NPC + b*128 + p
    xT = np.zeros((IN_C, TROWS), np.float16)
    nodes = np.arange(N)
    mm = nodes // NPC
    ll = nodes - mm * NPC
    cols = mm * RPC + ll  # block-major: b*128+p == l
    xT[:, cols] = x.T.astype(np.float16)

    consts = {
        "xT": xT,
        "W1sb": W1.astype(np.float16),                       # [128, 64]
        "W2sb": W2.astype(np.float16),                       # [64, 32]
        "b1b": np.tile(b1.astype(np.float32), (P, 1)),       # [128, 64]
        "b2b": np.tile(b2.astype(np.float32), (P, 1)),       # [128, 32]
        "iota": np.tile(np.arange(P, dtype=np.float16), (P, 1)),
        "ident": np.eye(P, dtype=np.float16),
    }
    return meta1, pc1, meta2, pc2, consts


# ---------------------------------------------------------------- bass build

def _dma_gather_raw(gpsimd, out_ap, in_ap, idxs_ap, num_idxs, elem_size, elem_step,
                    queue_num):
    nc = gpsimd
    mb = mybir
    stride_bytes_256 = exact_div(elem_step * mb.dt.size(in_ap.dtype), 256)
    _in_ap = nc.lower_ap_dma(in_ap, for_custom_bir_dma=True)
    _idxs_ap = nc.lower_ap(idxs_ap)
    _out_ap = nc.lower_ap(out_ap)
    return nc.add_instruction(
        mb.InstDMAGatherAnt(
            name=nc.bass.get_next_instruction_name(),
            ins=[*_in_ap, _idxs_ap, nc.lower_val_access(nc.to_reg(num_idxs))],
            outs=[_out_ap],
            transpose=False,
            num_idxs=num_idxs,
            elem_size=elem_size,
            stride_bytes_256=stride_bytes_256,
            gen_mode=0,
            single_packet=True,
            queue_num=queue_num,
            sbuf_tokens_per_rank=0,
            sbuf_free_dim_per_rank=0,
            sbuf_free_dim_pad_per_rank=0,
            sbuf_byte_offset=0,
        ))


def _win_rows(wi):
    return min(WIN, TROWS - wi * WIN)


def _emit_aggregation(nc, tc, meta, table, table_dep_insts, idx_t, dst_t, norm_t,
                      iota_t, acc, gpool, spool, ppool, feat, layer_tag, qoff=0):
    """Gather + one-hot-matmul aggregation for one layer.

    table: dram AP [TROWS, 128] fp16; feat: 64 (L1) or 32 (L2)
    acc: SBUF [128, NBLK, feat] fp32 (pre-zeroed)
    table_dep_insts: wi -> list of instructions the window's gathers must follow
    """
    wcalls = meta["wcalls"]
    gchunks = meta["gchunks"]
    gchunk_off = meta["gchunk_off"]
    chunk_base = meta["chunk_base"]
    call_base = meta["call_base"]
    q = qoff
    for wi in range(NWIN):
        rows = _win_rows(wi)
        win_ap = table[wi * WIN: wi * WIN + rows, :]
        # chunk -> (block, first, last) map for this window
        ends = {}
        for b in range(NBLK):
            c0 = int(gchunk_off[wi, b])
            c1 = c0 + int(gchunks[wi, b])
            ends[c0] = (b, c1)
        call_tiles = {}
        psum = None
        cur_end = None
        cur_b = None
        nchunks_w = wcalls[wi] * CHUNKS_PER_CALL
        for j in range(nchunks_w):
            cw = j // CHUNKS_PER_CALL
            if j % CHUNKS_PER_CALL == 0:
                g = gpool.tile([P, CHUNKS_PER_CALL, feat], mybir.dt.float16,
                               tag=f"g{layer_tag}")
                ci = call_base[wi] + cw
                gi = _dma_gather_raw(nc.gpsimd, g[:], win_ap,
                                     idx_t[:, ci * N16:(ci + 1) * N16],
                                     NI, feat, 128, queue_num=q % 4)
                q += 1
                for dep in table_dep_insts(wi):
                    add_dep_helper(gi.ins, dep, sync=True, reason="table RAW")
                call_tiles[cw] = g
            if j in ends:
                cur_b, cur_end = ends[j]
                psum = ppool.tile([P, feat], mybir.dt.float32, space="PSUM",
                                  tag="ps")
            if cur_end is None or j >= cur_end:
                continue  # tail padding chunks of the window (no group)
            J = chunk_base[wi] + j
            s = spool.tile([P, P], mybir.dt.float16, tag=f"s{layer_tag}")
            nc.vector.tensor_scalar(
                out=s[:], in0=iota_t[:],
                scalar1=dst_t[:, J:J + 1], scalar2=norm_t[:, J:J + 1],
                op0=mybir.AluOpType.is_equal, op1=mybir.AluOpType.mult)
            g = call_tiles[cw]
            start = j == cur_end - int(gchunks[wi, cur_b])
            stop = j == cur_end - 1
            nc.tensor.matmul(out=psum[:], lhsT=s[:], rhs=g[:, j % CHUNKS_PER_CALL, :],
                             start=start, stop=stop)
            if stop:
                nc.vector.tensor_add(out=acc[:, cur_b, :], in0=acc[:, cur_b, :],
                                     in1=psum[:])
    return q


def _build(meta1, meta2):
    nc = bacc.Bacc("TRN2", target_bir_lowering=False, debug=False,
                   num_devices=NCORE, num_swdge_queues=4)
    dt = mybir.dt
    ncalls1 = meta1["total_calls"]
    nch1 = meta1["total_chunks"]

    xT_d = nc.dram_tensor("xT", [IN_C, TROWS], dt.float16, kind="ExternalInput")
    W1_d = nc.dram_tensor("W1sb", [IN_C, HID], dt.float16, kind="ExternalInput")
    W2_d = nc.dram_tensor("W2sb", [HID, OUT_C], dt.float16, kind="ExternalInput")
    b1_d = nc.dram_tensor("b1b", [P, HID], dt.float32, kind="ExternalInput")
    b2_d = nc.dram_tensor("b2b", [P, OUT_C], dt.float32, kind="ExternalInput")
    iota_d = nc.dram_tensor("iota", [P, P], dt.float16, kind="ExternalInput")
    ident_d = nc.dram_tensor("ident", [P, P], dt.float16, kind="ExternalInput")
    idx1_d = nc.dram_tensor("idx1", [P, ncalls1 * N16], dt.int16, kind="ExternalInput")
    dst1_d = nc.dram_tensor("dst1", [P, nch1], dt.float32, kind="ExternalInput")
    norm1_d = nc.dram_tensor("norm1", [P, nch1], dt.float32, kind="ExternalInput")
    out_d = nc.dram_tensor("out", [RPC, OUT_C], dt.float32, kind="ExternalOutput")

    with tile.TileContext(nc) as tc:
        with (
            tc.tile_pool(name="dram", bufs=1, space="DRAM") as dram,
            tc.tile_pool(name="const", bufs=1) as cpool,
            tc.tile_pool(name="meta", bufs=1) as mpool,
            tc.tile_pool(name="xt", bufs=4) as xpool,
            tc.tile_pool(name="stage", bufs=2) as stpool,
            tc.tile_pool(name="g", bufs=6) as gpool,
            tc.tile_pool(name="s", bufs=8) as spool,
            tc.tile_pool(name="psA", bufs=5, space="PSUM") as ppoolA,
            tc.tile_pool(name="psB", bufs=3, space="PSUM") as ppoolB,
            tc.tile_pool(name="accp", bufs=1) as apool,
        ):
            table1 = dram.tile([TROWS, 128], dt.float16)
            table2 = dram.tile([TROWS, 128], dt.float16)
            bounce2 = dram.tile([RPC, 128], dt.float16)

            # ---- constants
            W1_t = cpool.tile([IN_C, HID], dt.float16)
            nc.sync.dma_start(W1_t[:], W1_d[:])
            W2_t = cpool.tile([HID, OUT_C], dt.float16)
            nc.sync.dma_start(W2_t[:], W2_d[:])
            b1_t = cpool.tile([P, HID], dt.float32)
            nc.sync.dma_start(b1_t[:], b1_d[:])
            b2_t = cpool.tile([P, OUT_C], dt.float32)
            nc.sync.dma_start(b2_t[:], b2_d[:])
            iota_t = cpool.tile([P, P], dt.float16)
            nc.sync.dma_start(iota_t[:], iota_d[:])
            ident_t = cpool.tile([P, P], dt.float16)
            nc.sync.dma_start(ident_t[:], ident_d[:])

            idx1_t = mpool.tile([P, ncalls1 * N16], dt.int16)
            nc.sync.dma_start(idx1_t[:], idx1_d[:])
            dst1_t = mpool.tile([P, nch1], dt.float32)
            nc.sync.dma_start(dst1_t[:], dst1_d[:])
            norm1_t = mpool.tile([P, nch1], dt.float32)
            nc.sync.dma_start(norm1_t[:], norm1_d[:])

            acc1 = apool.tile([P, NBLK, HID], dt.float32)
            nc.vector.memset(acc1[:], 0.0)
            acc2 = apool.tile([P, NBLK, OUT_C], dt.float32)
            nc.vector.memset(acc2[:], 0.0)

            # ---- layer-1 table: h1 = x @ W1 for ALL shards (recompute)
            table1_writes = []
            for mi in range(NCORE):
                stage = stpool.tile([P, NBLK, 128], dt.float16, tag="st1")
                for b in range(NBLK):
                    c0 = mi * RPC + b * P
                    xt = xpool.tile([IN_C, P], dt.float16)
                    nc.sync.dma_start(xt[:], xT_d[:, c0:c0 + P])
                    ps = ppoolB.tile([P, HID], dt.float32, space="PSUM", tag="aux")
                    nc.tensor.matmul(out=ps[:], lhsT=xt[:], rhs=W1_t[:],
                                     start=True, stop=True)
                    nc.vector.tensor_copy(out=stage[:, b, 0:HID], in_=ps[:])
                wr = nc.sync.dma_start(
                    table1[mi * RPC:(mi + 1) * RPC, :].rearrange(
                        "(p b) c -> p b c", b=NBLK),
                    stage[:])
                table1_writes.append(wr.ins)

            def t1_deps(wi):
                lo, hi = wi * WIN, wi * WIN + _win_rows(wi)
                return [table1_writes[mi] for mi in range(NCORE)
                        if mi * RPC < hi and (mi + 1) * RPC > lo]

            # ---- layer-1 aggregation
            _emit_aggregation(nc, tc, meta1, table1[:], t1_deps,
                              idx1_t, dst1_t, norm1_t, iota_t, acc1,
                              gpool, spool, ppoolA, HID, "1")

            # ---- layer-1 epilogue: h2 = relu(acc1 + b1); h2w = h2 @ W2
            h2all = apool.tile([P, NBLK, HID], dt.float16)
            b1b = b1_t[:, None, :].to_broadcast([P, NBLK, HID])
            nc.vector.tensor_tensor(out=h2all[:], in0=acc1[:], in1=b1b,
                                    op=mybir.AluOpType.add)
            nc.vector.tensor_scalar_max(out=h2all[:], in0=h2all[:], scalar1=0.0)

            h2wstage = apool.tile([P, NBLK, 128], dt.float16)
            for b in range(NBLK):
                pt = ppoolB.tile([HID, P], dt.float16, space="PSUM", tag="aux")
                nc.tensor.transpose(out=pt[:], in_=h2all[:, b, :], identity=ident_t[:])
                h2T = spool.tile([HID, P], dt.float16, tag="h2T")
                nc.vector.tensor_copy(out=h2T[:], in_=pt[:])
                pw = ppoolB.tile([P, OUT_C], dt.float32, space="PSUM", tag="aux")
                nc.tensor.matmul(out=pw[:], lhsT=h2T[:], rhs=W2_t[:],
                                 start=True, stop=True)
                nc.scalar.copy(out=h2wstage[:, b, 0:OUT_C], in_=pw[:])
            bw = nc.sync.dma_start(
                bounce2[:].rearrange("(p b) c -> p b c", b=NBLK), h2wstage[:])

            # ---- exchange h2w shards
            ag = nc.gpsimd.collective_compute(
                "AllGather", mybir.AluOpType.bypass,
                replica_groups=[list(range(NCORE))],
                ins=[bounce2.opt()], outs=[table2.opt()])
            add_dep_helper(ag.ins, bw.ins, sync=True, reason="bounce RAW")

            # ---- layer-2 metadata reuses the layer-1 tiles (same edges)
            def t2_deps(wi):
                return [ag.ins]

            _emit_aggregation(nc, tc, meta2, table2[:], t2_deps,
                              idx1_t, dst1_t, norm1_t, iota_t, acc2,
                              gpool, spool, ppoolA, OUT_C, "2")

            # ---- layer-2 epilogue: out = acc2 + b2
            outsb = apool.tile([P, NBLK, OUT_C], dt.float32)
            b2b = b2_t[:, None, :].to_broadcast([P, NBLK, OUT_C])
            nc.vector.tensor_tensor(out=outsb[:], in0=acc2[:], in1=b2b,
                                    op=mybir.AluOpType.add)
            nc.sync.dma_start(
                out_d[:].rearrange("(p b) c -> p b c", b=NBLK), outsb[:])

    nc.compile()
    return nc


# ---------------------------------------------------------------- entry point

def kernel(x, edge_index, W1, b1, W2, b2):
    x = np.asarray(x)
    edge_index = np.asarray(edge_index)
    W1 = np.asarray(W1, dtype=np.float32)
    b1 = np.asarray(b1, dtype=np.float32)
    W2 = np.asarray(W2, dtype=np.float32)
    b2 = np.asarray(b2, dtype=np.float32)

    meta1, pc1, meta2, pc2, consts = _host_prep(x, edge_index, W1, b1, W2, b2)

    key = (meta1["total_calls"], meta1["total_chunks"])
    if key not in _cache:
        _cache[key] = _build(meta1, meta2)
    nc = _cache[key]

    in_maps = []
    for mi in range(NCORE):
        wrapped, dstm, normm = pc1[mi]
        in_maps.append({
            "xT": consts["xT"], "W1sb": consts["W1sb"], "W2sb": consts["W2sb"],
            "b1b": consts["b1b"], "b2b": consts["b2b"],
            "iota": consts["iota"], "ident": consts["ident"],
            "idx1": wrapped, "dst1": dstm, "norm1": normm,
        })

    trace = os.environ.get("KERNEL_TRACE", "0") == "1"
    res = bass_utils.run_bass_kernel_spmd(
        nc, in_maps, core_ids=list(range(NCORE)), trace=trace)
    global last_exec_time_ns
    last_exec_time_ns = res.exec_time_ns

    out = np.empty((N, OUT_C), np.float32)
    ll = np.arange(NPC)
    rows = (ll % P) * NBLK + ll // P
    for mi in range(NCORE):
        out[mi * NPC:(mi + 1) * NPC] = res.results[mi]["out"][rows]
    return out


# revision 5
# speedup vs baseline: 1.2129x; 1.2129x over previous
"""GCN encoder (2-layer GCNConv) on 8 Trainium2 NeuronCores.

Strategy (graph/data parallel, edges partitioned by destination):
- Nodes sharded 12500/core. Layer tables (node features after the dense
  transform) live in each core's HBM at 256B row pitch, fp16.
- L1: every core recomputes h1 = x@W1 for ALL nodes (cheap, avoids a big
  collective); L2 exchanges the small transformed shard via AllGather.
- Message passing: per-edge rows fetched with the GPSIMD dma_gather ucode
  (4 SWDGE queues in parallel), aggregated per 128-dst block with one-hot
  matmuls (S = is_equal(iota, dst) * norm) accumulating in PSUM, flushed
  into an SBUF accumulator. Self-loops are ordinary edges.
"""
import os
import sys

sys.path.insert(0, '/opt/trn_rl_repo')

import numpy as np

import concourse.bass as bass
import concourse.bacc as bacc
import concourse.mybir as mybir
import concourse.tile as tile
from concourse import bass_utils
from concourse.bass import exact_div
from concourse.tile_rust import add_dep_helper

P = 128
N = 100000
NCORE = 8
NPC = N // NCORE            # 12500 nodes per core
NBLK = (NPC + P - 1) // P   # 98 blocks per core
RPC = NBLK * P              # 12544 table rows per core shard
TROWS = NCORE * RPC         # 100352
WIN = 32768                 # int16-addressable window
NWIN = (TROWS + WIN - 1) // WIN  # 4
IN_C = 128
HID = 64
OUT_C = 32
NI = 896                    # idxs per gather call (7 chunks; 57 ring descs)
CHUNKS_PER_CALL = NI // P   # 7
N16 = NI // 16              # 56

last_exec_time_ns = None
_cache = {}


# ---------------------------------------------------------------- host prep

def _table_row(n):
    """node id -> table row (p-major within each core shard)."""
    m = n // NPC
    l = n - m * NPC
    return m * RPC + (l % P) * NBLK + l // P


def _wrap_idx_batch(calls_idx):
    """[ncalls, NI] int16 -> [128, ncalls*N16] wrapped (16-part, replicated 8x)."""
    ncalls = calls_idx.shape[0]
    w = calls_idx.reshape(ncalls, N16, 16).transpose(0, 2, 1)  # [ncalls, 16, N16]
    w = np.tile(w, (1, 8, 1))                                   # [ncalls, 128, N16]
    return np.ascontiguousarray(w.transpose(1, 0, 2).reshape(P, ncalls * N16))


def _prep_layer(src_rows, dst, norm):
    """Build the common program structure + per-core padded edge data.

    src_rows: table row of each edge's source (int64)
    dst: destination node id (int64), norm: fp32
    Returns (meta, per_core_data).
    """
    m = dst // NPC
    l = dst - m * NPC
    blk = l // P
    dib = (l % P).astype(np.float16)
    w = src_rows // WIN
    widx = (src_rows - w * WIN).astype(np.int16)

    group = (m * NWIN + w) * NBLK + blk          # (core, window, block)
    counts = np.bincount(group, minlength=NCORE * NWIN * NBLK)
    counts = counts.reshape(NCORE, NWIN, NBLK)
    # common structure: max chunks over cores per (window, block)
    gchunks = np.maximum(1, -(-counts.max(axis=0) // P))   # [NWIN, NBLK] >=1
    # per-window chunk count padded to whole calls
    wchunks_raw = gchunks.sum(axis=1)                      # [NWIN]
    wcalls = -(-wchunks_raw // CHUNKS_PER_CALL)
    wchunks = wcalls * CHUNKS_PER_CALL
    total_chunks = int(wchunks.sum())
    total_calls = int(wcalls.sum())

    # group -> chunk offsets (window-local), groups ordered by block within window
    gchunk_off = np.zeros((NWIN, NBLK), np.int64)
    for wi in range(NWIN):
        gchunk_off[wi] = np.cumsum(gchunks[wi]) - gchunks[wi]
    chunk_base = np.cumsum(wchunks) - wchunks              # global chunk base per window
    call_base = np.cumsum(wcalls) - wcalls

    # place each edge into the padded layout (per core)
    order = np.lexsort((blk, w, m))
    m_s, w_s, blk_s, widx_s, dib_s, norm_s = (a[order] for a in (m, w, blk, widx, dib, norm))
    g_s = (m_s * NWIN + w_s) * NBLK + blk_s
    # position within its group
    gstart = np.zeros(NCORE * NWIN * NBLK + 1, np.int64)
    np.cumsum(np.bincount(g_s, minlength=NCORE * NWIN * NBLK), out=gstart[1:])
    within = np.arange(len(g_s)) - gstart[g_s]
    # padded slot (window-local edge index)
    slot = (gchunk_off[w_s, blk_s] * P + within).astype(np.int64)

    per_core = []
    for mi in range(NCORE):
        sel = m_s == mi
        idx_arr = np.zeros((NWIN, int(wchunks.max()) * P), np.int16)  # pad idx = 0 (valid row)
        dst_arr = np.zeros(total_chunks * P, np.float16)
        norm_arr = np.zeros(total_chunks * P, np.float16)
        sw, sslot = w_s[sel], slot[sel]
        idx_arr[sw, sslot] = widx_s[sel]
        gidx = (chunk_base[sw] * P + sslot)
        dst_arr[gidx] = dib_s[sel]
        norm_arr[gidx] = norm_s[sel]
        # idx calls: concat per window
        calls_idx = np.concatenate(
            [idx_arr[wi, : wchunks[wi] * P].reshape(-1, NI) for wi in range(NWIN)], axis=0)
        wrapped = _wrap_idx_batch(calls_idx)
        # [p, chunk] layout for dst/norm: edge i -> (p=i%128, chunk=i//128)
        dstm = np.ascontiguousarray(dst_arr.reshape(total_chunks, P).T)
        normm = np.ascontiguousarray(norm_arr.reshape(total_chunks, P).T)
        per_core.append((wrapped, dstm, normm))

    meta = {
        "wcalls": wcalls.tolist(),
        "wchunks": wchunks.tolist(),
        "gchunks": gchunks,
        "gchunk_off": gchunk_off,
        "chunk_base": chunk_base.tolist(),
        "call_base": call_base.tolist(),
        "total_chunks": total_chunks,
        "total_calls": total_calls,
    }
    return meta, per_core


def _host_prep(x, edge_index, W1, b1, W2, b2):
    src = edge_index[0].astype(np.int64)
    dst = edge_index[1].astype(np.int64)
    deg = np.bincount(dst, minlength=N).astype(np.float64) + 1.0
    dinv = 1.0 / np.sqrt(deg)
    # append self-loops as ordinary edges
    loops = np.arange(N, dtype=np.int64)
    src_a = np.concatenate([src, loops])
    dst_a = np.concatenate([dst, loops])
    norm_a = (dinv[src_a] * dinv[dst_a]).astype(np.float32)
    src_rows = _table_row(src_a)

    meta1, pc1 = _prep_layer(src_rows, dst_a, norm_a)
    meta2, pc2 = meta1, pc1  # same edges both layers

    # xT fp16 [128, TROWS] block-major columns: c = m*RPC + b*128 + p -> node m*NPC + b*128 + p
    xT = np.zeros((IN_C, TROWS), np.float16)
    nodes = np.arange(N)
    mm = nodes // NPC
    ll = nodes - mm * NPC
    cols = mm * RPC + ll  # block-major: b*128+p == l
    xT[:, cols] = x.T.astype(np.float16)

    consts = {
        "xT": xT,
        "W1sb": W1.astype(np.float16),                       # [128, 64]
        "W2sb": W2.astype(np.float16),                       # [64, 32]
        "b1b": np.tile(b1.astype(np.float32), (P, 1)),       # [128, 64]
        "b2b": np.tile(b2.astype(np.float32), (P, 1)),       # [128, 32]
        "iota": np.tile(np.arange(P, dtype=np.float16), (P, 1)),
        "ident": np.eye(P, dtype=np.float16),
    }
    return meta1, pc1, meta2, pc2, consts


# ---------------------------------------------------------------- bass build

def _dma_gather_raw(gpsimd, out_ap, in_ap, idxs_ap, num_idxs, elem_size, elem_step,
                    queue_num, ni_reg=None):
    nc = gpsimd
    mb = mybir
    stride_bytes_256 = exact_div(elem_step * mb.dt.size(in_ap.dtype), 256)
    _in_ap = nc.lower_ap_dma(in_ap, for_custom_bir_dma=True)
    _idxs_ap = nc.lower_ap(idxs_ap)
    _out_ap = nc.lower_ap(out_ap)
    return nc.add_instruction(
        mb.InstDMAGatherAnt(
            name=nc.bass.get_next_instruction_name(),
            ins=[*_in_ap, _idxs_ap, nc.lower_val_access(ni_reg if ni_reg is not None else nc.to_reg(num_idxs))],
            outs=[_out_ap],
            transpose=False,
            num_idxs=num_idxs,
            elem_size=elem_size,
            stride_bytes_256=stride_bytes_256,
            gen_mode=0,
            single_packet=True,
            queue_num=queue_num,
            sbuf_tokens_per_rank=0,
            sbuf_free_dim_per_rank=0,
            sbuf_free_dim_pad_per_rank=0,
            sbuf_byte_offset=0,
        ))


def _win_rows(wi):
    return min(WIN, TROWS - wi * WIN)


def _emit_aggregation(nc, tc, meta, table, table_dep_insts, idx_t, dst_t, norm_t,
                      iota_t, acc, gpool, spool, ppool, feat, layer_tag, ni_reg,
                      qoff=0):
    """Gather + one-hot-matmul aggregation for one layer.

    S built in SLAB-chunk batches with two broadcast tensor_tensor ops; PSUM
    banks hold 8 consecutive dst blocks, flushed into acc in one DVE add.
    """
    SLAB = 16
    wcalls = meta["wcalls"]
    gchunks = meta["gchunks"]
    gchunk_off = meta["gchunk_off"]
    chunk_base = meta["chunk_base"]
    call_base = meta["call_base"]
    q = qoff
    for wi in range(NWIN):
        rows = _win_rows(wi)
        win_ap = table[wi * WIN: wi * WIN + rows, :]
        ends = {}
        for b in range(NBLK):
            c0 = int(gchunk_off[wi, b])
            ends[c0] = (b, c0 + int(gchunks[wi, b]))
        call_tiles = {}
        slab = None
        bank = None
        bank_b0 = None
        bank_hi = None
        cur_end = None
        cur_b = None
        cur_c0 = None
        nchunks_w = wcalls[wi] * CHUNKS_PER_CALL

        def flush():
            nb = bank_hi - bank_b0 + 1
            nc.vector.tensor_tensor(
                out=acc[:, bank_b0:bank_b0 + nb, :],
                in0=acc[:, bank_b0:bank_b0 + nb, :],
                in1=bank[:, 0:nb * feat].rearrange("p (k f) -> p k f", f=feat),
                op=mybir.AluOpType.add)

        for j in range(nchunks_w):
            cw = j // CHUNKS_PER_CALL
            if j % CHUNKS_PER_CALL == 0:
                g = gpool.tile([P, CHUNKS_PER_CALL, feat], mybir.dt.float16,
                               tag=f"g{layer_tag}")
                ci = call_base[wi] + cw
                gi = _dma_gather_raw(nc.gpsimd, g[:], win_ap,
                                     idx_t[:, ci * N16:(ci + 1) * N16],
                                     NI, feat, 128, queue_num=q % 4, ni_reg=ni_reg)
                q += 1
                for dep in table_dep_insts(wi):
                    add_dep_helper(gi.ins, dep, sync=True, reason="table RAW")
                call_tiles[cw] = g
            if j % SLAB == 0:
                k = min(SLAB, nchunks_w - j)
                J = chunk_base[wi] + j
                slab = spool.tile([P, SLAB, P], mybir.dt.float16, tag="slab")
                iota_b = iota_t[:, None, :].to_broadcast([P, k, P])
                dst_b = dst_t[:, J:J + k][:, :, None].to_broadcast([P, k, P])
                norm_b = norm_t[:, J:J + k][:, :, None].to_broadcast([P, k, P])
                nc.vector.tensor_tensor(out=slab[:, 0:k, :], in0=iota_b, in1=dst_b,
                                        op=mybir.AluOpType.is_equal)
                nc.vector.tensor_tensor(out=slab[:, 0:k, :], in0=slab[:, 0:k, :],
                                        in1=norm_b, op=mybir.AluOpType.mult)
            if j in ends:
                cur_b, cur_end = ends[j]
                cur_c0 = j
                if bank is None or cur_b > bank_hi and cur_b >= bank_b0 + 8:
                    if bank is not None:
                        flush()
                    bank = ppool.tile([P, 8 * HID], mybir.dt.float32, space="PSUM",
                                      tag="ps")
                    bank_b0 = cur_b - (cur_b % 8)
                bank_hi = cur_b
            if cur_end is None or j >= cur_end:
                continue
            bslot = cur_b % 8
            nc.tensor.matmul(out=bank[:, bslot * feat:(bslot + 1) * feat],
                             lhsT=slab[:, j % SLAB, :],
                             rhs=call_tiles[cw][:, j % CHUNKS_PER_CALL, :],
                             start=(j == cur_c0), stop=(j == cur_end - 1))
        if bank is not None:
            flush()
    return q


def _build(meta1, meta2):
    nc = bacc.Bacc("TRN2", target_bir_lowering=False, debug=False,
                   num_devices=NCORE, num_swdge_queues=4)
    dt = mybir.dt
    ncalls1 = meta1["total_calls"]
    nch1 = meta1["total_chunks"]

    xT_d = nc.dram_tensor("xT", [IN_C, TROWS], dt.float16, kind="ExternalInput")
    W1_d = nc.dram_tensor("W1sb", [IN_C, HID], dt.float16, kind="ExternalInput")
    W2_d = nc.dram_tensor("W2sb", [HID, OUT_C], dt.float16, kind="ExternalInput")
    b1_d = nc.dram_tensor("b1b", [P, HID], dt.float32, kind="ExternalInput")
    b2_d = nc.dram_tensor("b2b", [P, OUT_C], dt.float32, kind="ExternalInput")
    iota_d = nc.dram_tensor("iota", [P, P], dt.float16, kind="ExternalInput")
    ident_d = nc.dram_tensor("ident", [P, P], dt.float16, kind="ExternalInput")
    idx1_d = nc.dram_tensor("idx1", [P, ncalls1 * N16], dt.int16, kind="ExternalInput")
    dst1_d = nc.dram_tensor("dst1", [P, nch1], dt.float16, kind="ExternalInput")
    norm1_d = nc.dram_tensor("norm1", [P, nch1], dt.float16, kind="ExternalInput")
    out_d = nc.dram_tensor("out", [RPC, OUT_C], dt.float32, kind="ExternalOutput")

    with tile.TileContext(nc) as tc:
        with (
            tc.tile_pool(name="dram", bufs=1, space="DRAM") as dram,
            tc.tile_pool(name="const", bufs=1) as cpool,
            tc.tile_pool(name="meta", bufs=1) as mpool,
            tc.tile_pool(name="xt", bufs=4) as xpool,
            tc.tile_pool(name="stage", bufs=2) as stpool,
            tc.tile_pool(name="g", bufs=6) as gpool,
            tc.tile_pool(name="s", bufs=3) as spool,
            tc.tile_pool(name="psA", bufs=4, space="PSUM") as ppoolA,
            tc.tile_pool(name="psB", bufs=3, space="PSUM") as ppoolB,
            tc.tile_pool(name="accp", bufs=1) as apool,
        ):
            table1 = dram.tile([TROWS, 128], dt.float16)
            table2 = dram.tile([TROWS, 128], dt.float16)
            bounce2 = dram.tile([RPC, 128], dt.float16)

            # ---- constants
            W1_t = cpool.tile([IN_C, HID], dt.float16)
            nc.sync.dma_start(W1_t[:], W1_d[:])
            W2_t = cpool.tile([HID, OUT_C], dt.float16)
            nc.sync.dma_start(W2_t[:], W2_d[:])
            b1_t = cpool.tile([P, HID], dt.float32)
            nc.sync.dma_start(b1_t[:], b1_d[:])
            b2_t = cpool.tile([P, OUT_C], dt.float32)
            nc.sync.dma_start(b2_t[:], b2_d[:])
            iota_t = cpool.tile([P, P], dt.float16)
            nc.sync.dma_start(iota_t[:], iota_d[:])
            ident_t = cpool.tile([P, P], dt.float16)
            nc.sync.dma_start(ident_t[:], ident_d[:])

            idx1_t = mpool.tile([P, ncalls1 * N16], dt.int16)
            nc.sync.dma_start(idx1_t[:], idx1_d[:])
            dst1_t = mpool.tile([P, nch1], dt.float16)
            nc.sync.dma_start(dst1_t[:], dst1_d[:])
            norm1_t = mpool.tile([P, nch1], dt.float16)
            nc.sync.dma_start(norm1_t[:], norm1_d[:])

            ni_reg = nc.gpsimd.to_reg(NI)
            acc1 = apool.tile([P, NBLK, HID], dt.float32)
            nc.vector.memset(acc1[:], 0.0)
            acc2 = apool.tile([P, NBLK, OUT_C], dt.float32)
            nc.vector.memset(acc2[:], 0.0)

            # ---- layer-1 table: h1 = x @ W1 for ALL shards (recompute)
            table1_writes = []
            for mi in range(NCORE):
                stage = stpool.tile([P, NBLK, 128], dt.float16, tag="st1")
                for s0 in range(0, NBLK, 8):
                    nb = min(8, NBLK - s0)
                    c0 = mi * RPC + s0 * P
                    xt = xpool.tile([IN_C, 8 * P], dt.float16)
                    nc.sync.dma_start(xt[:, 0:nb * P], xT_d[:, c0:c0 + nb * P])
                    ps = ppoolB.tile([P, 8 * HID], dt.float32, space="PSUM", tag="aux")
                    for k in range(nb):
                        nc.tensor.matmul(out=ps[:, k * HID:(k + 1) * HID],
                                         lhsT=xt[:, k * P:(k + 1) * P], rhs=W1_t[:],
                                         start=True, stop=True)
                    nc.vector.tensor_copy(
                        out=stage[:, s0:s0 + nb, 0:HID],
                        in_=ps[:, 0:nb * HID].rearrange("p (k f) -> p k f", f=HID))
                wr = nc.sync.dma_start(
                    table1[mi * RPC:(mi + 1) * RPC, :].rearrange(
                        "(p b) c -> p b c", b=NBLK),
                    stage[:])
                table1_writes.append(wr.ins)

            def t1_deps(wi):
                lo, hi = wi * WIN, wi * WIN + _win_rows(wi)
                return [table1_writes[mi] for mi in range(NCORE)
                        if mi * RPC < hi and (mi + 1) * RPC > lo]

            # ---- layer-1 aggregation
            _emit_aggregation(nc, tc, meta1, table1[:], t1_deps,
                              idx1_t, dst1_t, norm1_t, iota_t, acc1,
                              gpool, spool, ppoolA, HID, "1", ni_reg)

            # ---- layer-1 epilogue: h2 = relu(acc1 + b1); h2w = h2 @ W2
            h2all = apool.tile([P, NBLK, HID], dt.float16)
            b1b = b1_t[:, None, :].to_broadcast([P, NBLK, HID])
            nc.vector.tensor_tensor(out=h2all[:], in0=acc1[:], in1=b1b,
                                    op=mybir.AluOpType.add)
            nc.vector.tensor_scalar_max(out=h2all[:], in0=h2all[:], scalar1=0.0)

            h2wstage = apool.tile([P, NBLK, 128], dt.float16)
            for b in range(NBLK):
                pt = ppoolB.tile([HID, P], dt.float16, space="PSUM", tag="aux")
                nc.tensor.transpose(out=pt[:], in_=h2all[:, b, :], identity=ident_t[:])
                h2T = spool.tile([HID, P], dt.float16, tag="h2T")
                nc.vector.tensor_copy(out=h2T[:], in_=pt[:])
                pw = ppoolB.tile([P, OUT_C], dt.float32, space="PSUM", tag="aux")
                nc.tensor.matmul(out=pw[:], lhsT=h2T[:], rhs=W2_t[:],
                                 start=True, stop=True)
                nc.scalar.copy(out=h2wstage[:, b, 0:OUT_C], in_=pw[:])
            bw = nc.sync.dma_start(
                bounce2[:].rearrange("(p b) c -> p b c", b=NBLK), h2wstage[:])

            # ---- exchange h2w shards
            ag = nc.gpsimd.collective_compute(
                "AllGather", mybir.AluOpType.bypass,
                replica_groups=[list(range(NCORE))],
                ins=[bounce2.opt()], outs=[table2.opt()])
            add_dep_helper(ag.ins, bw.ins, sync=True, reason="bounce RAW")

            # ---- layer-2 metadata reuses the layer-1 tiles (same edges)
            def t2_deps(wi):
                return [ag.ins]

            _emit_aggregation(nc, tc, meta2, table2[:], t2_deps,
                              idx1_t, dst1_t, norm1_t, iota_t, acc2,
                              gpool, spool, ppoolA, OUT_C, "2", ni_reg)

            # ---- layer-2 epilogue: out = acc2 + b2
            outsb = apool.tile([P, NBLK, OUT_C], dt.float32)
            b2b = b2_t[:, None, :].to_broadcast([P, NBLK, OUT_C])
            nc.vector.tensor_tensor(out=outsb[:], in0=acc2[:], in1=b2b,
                                    op=mybir.AluOpType.add)
            nc.sync.dma_start(
                out_d[:].rearrange("(p b) c -> p b c", b=NBLK), outsb[:])

    nc.compile()
    return nc


# ---------------------------------------------------------------- entry point

def kernel(x, edge_index, W1, b1, W2, b2):
    x = np.asarray(x)
    edge_index = np.asarray(edge_index)
    W1 = np.asarray(W1, dtype=np.float32)
    b1 = np.asarray(b1, dtype=np.float32)
    W2 = np.asarray(W2, dtype=np.float32)
    b2 = np.asarray(b2, dtype=np.float32)

    meta1, pc1, meta2, pc2, consts = _host_prep(x, edge_index, W1, b1, W2, b2)

    key = (meta1["total_calls"], meta1["total_chunks"])
    if key not in _cache:
        _cache[key] = _build(meta1, meta2)
    nc = _cache[key]

    in_maps = []
    for mi in range(NCORE):
        wrapped, dstm, normm = pc1[mi]
        in_maps.append({
            "xT": consts["xT"], "W1sb": consts["W1sb"], "W2sb": consts["W2sb"],
            "b1b": consts["b1b"], "b2b": consts["b2b"],
            "iota": consts["iota"], "ident": consts["ident"],
            "idx1": wrapped, "dst1": dstm, "norm1": normm,
        })

    trace = os.environ.get("KERNEL_TRACE", "0") == "1"
    res = bass_utils.run_bass_kernel_spmd(
        nc, in_maps, core_ids=list(range(NCORE)), trace=trace)
    global last_exec_time_ns
    last_exec_time_ns = res.exec_time_ns

    out = np.empty((N, OUT_C), np.float32)
    ll = np.arange(NPC)
    rows = (ll % P) * NBLK + ll // P
    for mi in range(NCORE):
        out[mi * NPC:(mi + 1) * NPC] = res.results[mi]["out"][rows]
    return out


# revision 7
# speedup vs baseline: 1.2370x; 1.0199x over previous
"""GCN encoder (2-layer GCNConv) on 8 Trainium2 NeuronCores.

Strategy (graph/data parallel, edges partitioned by destination):
- Nodes sharded 12500/core. Layer tables (node features after the dense
  transform) live in each core's HBM at 256B row pitch, fp16.
- L1: every core recomputes h1 = x@W1 for ALL nodes (cheap, avoids a big
  collective); L2 exchanges the small transformed shard via AllGather.
- Message passing: per-edge rows fetched with the GPSIMD dma_gather ucode
  (4 SWDGE queues in parallel), aggregated per 128-dst block with one-hot
  matmuls (S = is_equal(iota, dst) * norm) accumulating in PSUM, flushed
  into an SBUF accumulator. Self-loops are ordinary edges.
"""
import os
import sys

sys.path.insert(0, '/opt/trn_rl_repo')

import numpy as np

import concourse.bass as bass
import concourse.bacc as bacc
import concourse.mybir as mybir
import concourse.tile as tile
from concourse import bass_utils
from concourse.bass import exact_div
from concourse.tile_rust import add_dep_helper

P = 128
N = 100000
NCORE = 8
NPC = N // NCORE            # 12500 nodes per core
NBLK = (NPC + P - 1) // P   # 98 blocks per core
RPC = NBLK * P              # 12544 table rows per core shard
TROWS = NCORE * RPC         # 100352
WIN = 32768                 # int16-addressable window
NWIN = (TROWS + WIN - 1) // WIN  # 4
IN_C = 128
HID = 64
OUT_C = 32
NI = 896                    # idxs per gather call (7 chunks; 57 ring descs)
CHUNKS_PER_CALL = NI // P   # 7
N16 = NI // 16              # 56

last_exec_time_ns = None
_cache = {}


# ---------------------------------------------------------------- host prep

def _table_row(n):
    """node id -> table row (p-major within each core shard)."""
    m = n // NPC
    l = n - m * NPC
    return m * RPC + (l % P) * NBLK + l // P


def _wrap_idx_batch(calls_idx):
    """[ncalls, NI] int16 -> [128, ncalls*N16] wrapped (16-part, replicated 8x)."""
    ncalls = calls_idx.shape[0]
    w = calls_idx.reshape(ncalls, N16, 16).transpose(0, 2, 1)  # [ncalls, 16, N16]
    w = np.tile(w, (1, 8, 1))                                   # [ncalls, 128, N16]
    return np.ascontiguousarray(w.transpose(1, 0, 2).reshape(P, ncalls * N16))


def _prep_layer(src_rows, dst, norm):
    """Build the common program structure + per-core padded edge data.

    src_rows: table row of each edge's source (int64)
    dst: destination node id (int64), norm: fp32
    Returns (meta, per_core_data).
    """
    m = dst // NPC
    l = dst - m * NPC
    blk = l // P
    dib = (l % P).astype(np.float16)
    w = src_rows // WIN
    widx = (src_rows - w * WIN).astype(np.int16)

    group = (m * NWIN + w) * NBLK + blk          # (core, window, block)
    counts = np.bincount(group, minlength=NCORE * NWIN * NBLK)
    counts = counts.reshape(NCORE, NWIN, NBLK)
    # common structure: max chunks over cores per (window, block)
    gchunks = np.maximum(1, -(-counts.max(axis=0) // P))   # [NWIN, NBLK] >=1
    # per-window chunk count padded to whole calls
    wchunks_raw = gchunks.sum(axis=1)                      # [NWIN]
    wcalls = -(-wchunks_raw // CHUNKS_PER_CALL)
    wchunks = wcalls * CHUNKS_PER_CALL
    total_chunks = int(wchunks.sum())
    total_calls = int(wcalls.sum())

    # group -> chunk offsets (window-local), groups ordered by block within window
    gchunk_off = np.zeros((NWIN, NBLK), np.int64)
    for wi in range(NWIN):
        gchunk_off[wi] = np.cumsum(gchunks[wi]) - gchunks[wi]
    chunk_base = np.cumsum(wchunks) - wchunks              # global chunk base per window
    call_base = np.cumsum(wcalls) - wcalls

    # place each edge into the padded layout (per core)
    order = np.lexsort((blk, w, m))
    m_s, w_s, blk_s, widx_s, dib_s, norm_s = (a[order] for a in (m, w, blk, widx, dib, norm))
    g_s = (m_s * NWIN + w_s) * NBLK + blk_s
    # position within its group
    gstart = np.zeros(NCORE * NWIN * NBLK + 1, np.int64)
    np.cumsum(np.bincount(g_s, minlength=NCORE * NWIN * NBLK), out=gstart[1:])
    within = np.arange(len(g_s)) - gstart[g_s]
    # padded slot (window-local edge index)
    slot = (gchunk_off[w_s, blk_s] * P + within).astype(np.int64)

    per_core = []
    for mi in range(NCORE):
        sel = m_s == mi
        idx_arr = np.zeros((NWIN, int(wchunks.max()) * P), np.int16)  # pad idx = 0 (valid row)
        dst_arr = np.zeros(total_chunks * P, np.float16)
        norm_arr = np.zeros(total_chunks * P, np.float16)
        sw, sslot = w_s[sel], slot[sel]
        idx_arr[sw, sslot] = widx_s[sel]
        gidx = (chunk_base[sw] * P + sslot)
        dst_arr[gidx] = dib_s[sel]
        norm_arr[gidx] = norm_s[sel]
        # idx calls: concat per window
        calls_idx = np.concatenate(
            [idx_arr[wi, : wchunks[wi] * P].reshape(-1, NI) for wi in range(NWIN)], axis=0)
        wrapped = _wrap_idx_batch(calls_idx)
        # [p, chunk] layout for dst/norm: edge i -> (p=i%128, chunk=i//128)
        dstm = np.ascontiguousarray(dst_arr.reshape(total_chunks, P).T)
        normm = np.ascontiguousarray(norm_arr.reshape(total_chunks, P).T)
        per_core.append((wrapped, dstm, normm))

    meta = {
        "wcalls": wcalls.tolist(),
        "wchunks": wchunks.tolist(),
        "gchunks": gchunks,
        "gchunk_off": gchunk_off,
        "chunk_base": chunk_base.tolist(),
        "call_base": call_base.tolist(),
        "total_chunks": total_chunks,
        "total_calls": total_calls,
    }
    return meta, per_core


def _host_prep(x, edge_index, W1, b1, W2, b2):
    src = edge_index[0].astype(np.int64)
    dst = edge_index[1].astype(np.int64)
    deg = np.bincount(dst, minlength=N).astype(np.float64) + 1.0
    dinv = 1.0 / np.sqrt(deg)
    # append self-loops as ordinary edges
    loops = np.arange(N, dtype=np.int64)
    src_a = np.concatenate([src, loops])
    dst_a = np.concatenate([dst, loops])
    norm_a = (dinv[src_a] * dinv[dst_a]).astype(np.float32)
    src_rows = _table_row(src_a)

    meta1, pc1 = _prep_layer(src_rows, dst_a, norm_a)
    meta2, pc2 = meta1, pc1  # same edges both layers

    # xT fp16 [128, TROWS] block-major columns: c = m*RPC + b*128 + p -> node m*NPC + b*128 + p
    xT = np.zeros((IN_C, TROWS), np.float16)
    nodes = np.arange(N)
    mm = nodes // NPC
    ll = nodes - mm * NPC
    cols = mm * RPC + ll  # block-major: b*128+p == l
    xT[:, cols] = x.T.astype(np.float16)

    consts = {
        "xT": xT,
        "W1sb": W1.astype(np.float16),                       # [128, 64]
        "W2sb": W2.astype(np.float16),                       # [64, 32]
        "b1b": np.tile(b1.astype(np.float32), (P, 1)),       # [128, 64]
        "b2b": np.tile(b2.astype(np.float32), (P, 1)),       # [128, 32]
        "iota": np.tile(np.arange(P, dtype=np.float16), (P, 1)),
        "ident": np.eye(P, dtype=np.float16),
    }
    return meta1, pc1, meta2, pc2, consts


# ---------------------------------------------------------------- bass build

def _dma_gather_raw(gpsimd, out_ap, in_ap, idxs_ap, num_idxs, elem_size, elem_step,
                    queue_num, ni_reg=None):
    nc = gpsimd
    mb = mybir
    stride_bytes_256 = exact_div(elem_step * mb.dt.size(in_ap.dtype), 256)
    _in_ap = nc.lower_ap_dma(in_ap, for_custom_bir_dma=True)
    _idxs_ap = nc.lower_ap(idxs_ap)
    _out_ap = nc.lower_ap(out_ap)
    return nc.add_instruction(
        mb.InstDMAGatherAnt(
            name=nc.bass.get_next_instruction_name(),
            ins=[*_in_ap, _idxs_ap, nc.lower_val_access(ni_reg if ni_reg is not None else nc.to_reg(num_idxs))],
            outs=[_out_ap],
            transpose=False,
            num_idxs=num_idxs,
            elem_size=elem_size,
            stride_bytes_256=stride_bytes_256,
            gen_mode=0,
            single_packet=True,
            queue_num=queue_num,
            sbuf_tokens_per_rank=0,
            sbuf_free_dim_per_rank=0,
            sbuf_free_dim_pad_per_rank=0,
            sbuf_byte_offset=0,
        ))


def _win_rows(wi):
    return min(WIN, TROWS - wi * WIN)


def _emit_aggregation(nc, tc, meta, table, table_dep_insts, idx_t, dst_t, norm_t,
                      iota_t, acc, gpool, spool, ppool, feat, layer_tag, ni_reg,
                      qoff=0):
    """Gather + one-hot-matmul aggregation for one layer.

    S built in SLAB-chunk batches with two broadcast tensor_tensor ops; PSUM
    banks hold 8 consecutive dst blocks, flushed into acc in one DVE add.
    """
    SLAB = 16
    wcalls = meta["wcalls"]
    gchunks = meta["gchunks"]
    gchunk_off = meta["gchunk_off"]
    chunk_base = meta["chunk_base"]
    call_base = meta["call_base"]
    q = qoff
    for wi in range(NWIN):
        rows = _win_rows(wi)
        win_ap = table[wi * WIN: wi * WIN + rows, :]
        ends = {}
        for b in range(NBLK):
            c0 = int(gchunk_off[wi, b])
            ends[c0] = (b, c0 + int(gchunks[wi, b]))
        call_tiles = {}
        slab = None
        bank = None
        bank_b0 = None
        bank_hi = None
        cur_end = None
        cur_b = None
        cur_c0 = None
        nchunks_w = wcalls[wi] * CHUNKS_PER_CALL
        pending = []   # deferred (bank, b0, hi) flushes so slab builds never
                       # queue behind a flush that waits on in-flight matmuls

        def emit_flush(entry):
            fbank, fb0, fhi = entry
            nb = fhi - fb0 + 1
            nc.vector.tensor_tensor(
                out=acc[:, fb0:fb0 + nb, :],
                in0=acc[:, fb0:fb0 + nb, :],
                in1=fbank[:, 0:nb * feat].rearrange("p (k f) -> p k f", f=feat),
                op=mybir.AluOpType.add)

        for j in range(nchunks_w):
            cw = j // CHUNKS_PER_CALL
            if j % CHUNKS_PER_CALL == 0:
                g = gpool.tile([P, CHUNKS_PER_CALL, feat], mybir.dt.float16,
                               tag=f"g{layer_tag}")
                ci = call_base[wi] + cw
                gi = _dma_gather_raw(nc.gpsimd, g[:], win_ap,
                                     idx_t[:, ci * N16:(ci + 1) * N16],
                                     NI, feat, 128, queue_num=q % 4, ni_reg=ni_reg)
                q += 1
                for dep in table_dep_insts(wi):
                    add_dep_helper(gi.ins, dep, sync=True, reason="table RAW")
                call_tiles[cw] = g
            if j % SLAB == 0:
                k = min(SLAB, nchunks_w - j)
                J = chunk_base[wi] + j
                slab = spool.tile([P, SLAB, P], mybir.dt.float16, tag="slab")
                iota_b = iota_t[:, None, :].to_broadcast([P, k, P])
                dst_b = dst_t[:, J:J + k][:, :, None].to_broadcast([P, k, P])
                norm_b = norm_t[:, J:J + k][:, :, None].to_broadcast([P, k, P])
                nc.vector.tensor_tensor(out=slab[:, 0:k, :], in0=iota_b, in1=dst_b,
                                        op=mybir.AluOpType.is_equal)
                nc.vector.tensor_tensor(out=slab[:, 0:k, :], in0=slab[:, 0:k, :],
                                        in1=norm_b, op=mybir.AluOpType.mult)
            if j in ends:
                cur_b, cur_end = ends[j]
                cur_c0 = j
                if bank is None or cur_b > bank_hi and cur_b >= bank_b0 + 8:
                    if bank is not None:
                        pending.append((bank, bank_b0, bank_hi))
                        if len(pending) > 2:
                            emit_flush(pending.pop(0))
                    bank = ppool.tile([P, 8 * HID], mybir.dt.float32, space="PSUM",
                                      tag="ps")
                    bank_b0 = cur_b - (cur_b % 8)
                bank_hi = cur_b
            if cur_end is None or j >= cur_end:
                continue
            bslot = cur_b % 8
            nc.tensor.matmul(out=bank[:, bslot * feat:(bslot + 1) * feat],
                             lhsT=slab[:, j % SLAB, :],
                             rhs=call_tiles[cw][:, j % CHUNKS_PER_CALL, :],
                             start=(j == cur_c0), stop=(j == cur_end - 1))
        if bank is not None:
            pending.append((bank, bank_b0, bank_hi))
        for e in pending:
            emit_flush(e)
    return q


def _build(meta1, meta2):
    nc = bacc.Bacc("TRN2", target_bir_lowering=False, debug=False,
                   num_devices=NCORE, num_swdge_queues=4)
    dt = mybir.dt
    ncalls1 = meta1["total_calls"]
    nch1 = meta1["total_chunks"]

    xT_d = nc.dram_tensor("xT", [IN_C, TROWS], dt.float16, kind="ExternalInput")
    W1_d = nc.dram_tensor("W1sb", [IN_C, HID], dt.float16, kind="ExternalInput")
    W2_d = nc.dram_tensor("W2sb", [HID, OUT_C], dt.float16, kind="ExternalInput")
    b1_d = nc.dram_tensor("b1b", [P, HID], dt.float32, kind="ExternalInput")
    b2_d = nc.dram_tensor("b2b", [P, OUT_C], dt.float32, kind="ExternalInput")
    iota_d = nc.dram_tensor("iota", [P, P], dt.float16, kind="ExternalInput")
    ident_d = nc.dram_tensor("ident", [P, P], dt.float16, kind="ExternalInput")
    idx1_d = nc.dram_tensor("idx1", [P, ncalls1 * N16], dt.int16, kind="ExternalInput")
    dst1_d = nc.dram_tensor("dst1", [P, nch1], dt.float16, kind="ExternalInput")
    norm1_d = nc.dram_tensor("norm1", [P, nch1], dt.float16, kind="ExternalInput")
    out_d = nc.dram_tensor("out", [RPC, OUT_C], dt.float32, kind="ExternalOutput")

    with tile.TileContext(nc) as tc:
        with (
            tc.tile_pool(name="dram", bufs=1, space="DRAM") as dram,
            tc.tile_pool(name="const", bufs=1) as cpool,
            tc.tile_pool(name="meta", bufs=1) as mpool,
            tc.tile_pool(name="xt", bufs=4) as xpool,
            tc.tile_pool(name="stage", bufs=2) as stpool,
            tc.tile_pool(name="g", bufs=10) as gpool,
            tc.tile_pool(name="s", bufs=4) as spool,
            tc.tile_pool(name="psA", bufs=5, space="PSUM") as ppoolA,
            tc.tile_pool(name="psB", bufs=3, space="PSUM") as ppoolB,
            tc.tile_pool(name="accp", bufs=1) as apool,
        ):
            table1 = dram.tile([TROWS, 128], dt.float16)
            table2 = dram.tile([TROWS, 128], dt.float16)
            bounce2 = dram.tile([RPC, 128], dt.float16)

            # ---- constants
            W1_t = cpool.tile([IN_C, HID], dt.float16)
            nc.sync.dma_start(W1_t[:], W1_d[:])
            W2_t = cpool.tile([HID, OUT_C], dt.float16)
            nc.sync.dma_start(W2_t[:], W2_d[:])
            b1_t = cpool.tile([P, HID], dt.float32)
            nc.sync.dma_start(b1_t[:], b1_d[:])
            b2_t = cpool.tile([P, OUT_C], dt.float32)
            nc.sync.dma_start(b2_t[:], b2_d[:])
            iota_t = cpool.tile([P, P], dt.float16)
            nc.sync.dma_start(iota_t[:], iota_d[:])
            ident_t = cpool.tile([P, P], dt.float16)
            nc.sync.dma_start(ident_t[:], ident_d[:])

            idx1_t = mpool.tile([P, ncalls1 * N16], dt.int16)
            nc.sync.dma_start(idx1_t[:], idx1_d[:])
            dst1_t = mpool.tile([P, nch1], dt.float16)
            nc.sync.dma_start(dst1_t[:], dst1_d[:])
            norm1_t = mpool.tile([P, nch1], dt.float16)
            nc.sync.dma_start(norm1_t[:], norm1_d[:])

            ni_reg = nc.gpsimd.to_reg(NI)
            acc1 = apool.tile([P, NBLK, HID], dt.float32)
            nc.vector.memset(acc1[:], 0.0)
            acc2 = apool.tile([P, NBLK, OUT_C], dt.float32)
            nc.vector.memset(acc2[:], 0.0)

            # ---- layer-1 table: h1 = x @ W1 for ALL shards (recompute)
            table1_writes = []
            for mi in range(NCORE):
                stage = stpool.tile([P, NBLK, 128], dt.float16, tag="st1")
                for s0 in range(0, NBLK, 8):
                    nb = min(8, NBLK - s0)
                    c0 = mi * RPC + s0 * P
                    xt = xpool.tile([IN_C, 8 * P], dt.float16)
                    nc.sync.dma_start(xt[:, 0:nb * P], xT_d[:, c0:c0 + nb * P])
                    ps = ppoolB.tile([P, 8 * HID], dt.float32, space="PSUM", tag="aux")
                    for k in range(nb):
                        nc.tensor.matmul(out=ps[:, k * HID:(k + 1) * HID],
                                         lhsT=xt[:, k * P:(k + 1) * P], rhs=W1_t[:],
                                         start=True, stop=True)
                    nc.vector.tensor_copy(
                        out=stage[:, s0:s0 + nb, 0:HID],
                        in_=ps[:, 0:nb * HID].rearrange("p (k f) -> p k f", f=HID))
                wr = nc.sync.dma_start(
                    table1[mi * RPC:(mi + 1) * RPC, :].rearrange(
                        "(p b) c -> p b c", b=NBLK),
                    stage[:])
                table1_writes.append(wr.ins)

            def t1_deps(wi):
                lo, hi = wi * WIN, wi * WIN + _win_rows(wi)
                return [table1_writes[mi] for mi in range(NCORE)
                        if mi * RPC < hi and (mi + 1) * RPC > lo]

            # ---- layer-1 aggregation
            _emit_aggregation(nc, tc, meta1, table1[:], t1_deps,
                              idx1_t, dst1_t, norm1_t, iota_t, acc1,
                              gpool, spool, ppoolA, HID, "1", ni_reg)

            # ---- layer-1 epilogue: h2 = relu(acc1 + b1); h2w = h2 @ W2
            h2all = apool.tile([P, NBLK, HID], dt.float16)
            b1b = b1_t[:, None, :].to_broadcast([P, NBLK, HID])
            nc.vector.tensor_tensor(out=h2all[:], in0=acc1[:], in1=b1b,
                                    op=mybir.AluOpType.add)
            nc.vector.tensor_scalar_max(out=h2all[:], in0=h2all[:], scalar1=0.0)

            h2wstage = apool.tile([P, NBLK, OUT_C], dt.float16)
            for b in range(NBLK):
                pt = ppoolB.tile([HID, P], dt.float16, space="PSUM", tag="aux")
                nc.tensor.transpose(out=pt[:], in_=h2all[:, b, :], identity=ident_t[:])
                h2T = spool.tile([HID, P], dt.float16, tag="h2T")
                nc.vector.tensor_copy(out=h2T[:], in_=pt[:])
                pw = ppoolB.tile([P, OUT_C], dt.float32, space="PSUM", tag="aux")
                nc.tensor.matmul(out=pw[:], lhsT=h2T[:], rhs=W2_t[:],
                                 start=True, stop=True)
                nc.scalar.copy(out=h2wstage[:, b, :], in_=pw[:])
            bw = nc.sync.dma_start(
                bounce2[:].rearrange("(p b) c -> p b c", b=NBLK)[:, :, 0:OUT_C],
                h2wstage[:])

            # ---- exchange h2w shards
            ag = nc.gpsimd.collective_compute(
                "AllGather", mybir.AluOpType.bypass,
                replica_groups=[list(range(NCORE))],
                ins=[bounce2.opt()], outs=[table2.opt()])
            add_dep_helper(ag.ins, bw.ins, sync=True, reason="bounce RAW")

            # ---- layer-2 metadata reuses the layer-1 tiles (same edges)
            def t2_deps(wi):
                return [ag.ins]

            _emit_aggregation(nc, tc, meta2, table2[:], t2_deps,
                              idx1_t, dst1_t, norm1_t, iota_t, acc2,
                              gpool, spool, ppoolA, OUT_C, "2", ni_reg)

            # ---- layer-2 epilogue: out = acc2 + b2 (in place)
            b2b = b2_t[:, None, :].to_broadcast([P, NBLK, OUT_C])
            nc.vector.tensor_tensor(out=acc2[:], in0=acc2[:], in1=b2b,
                                    op=mybir.AluOpType.add)
            nc.sync.dma_start(
                out_d[:].rearrange("(p b) c -> p b c", b=NBLK), acc2[:])

    nc.compile()
    return nc


# ---------------------------------------------------------------- entry point

def kernel(x, edge_index, W1, b1, W2, b2):
    x = np.asarray(x)
    edge_index = np.asarray(edge_index)
    W1 = np.asarray(W1, dtype=np.float32)
    b1 = np.asarray(b1, dtype=np.float32)
    W2 = np.asarray(W2, dtype=np.float32)
    b2 = np.asarray(b2, dtype=np.float32)

    meta1, pc1, meta2, pc2, consts = _host_prep(x, edge_index, W1, b1, W2, b2)

    key = (meta1["total_calls"], meta1["total_chunks"])
    if key not in _cache:
        _cache[key] = _build(meta1, meta2)
    nc = _cache[key]

    in_maps = []
    for mi in range(NCORE):
        wrapped, dstm, normm = pc1[mi]
        in_maps.append({
            "xT": consts["xT"], "W1sb": consts["W1sb"], "W2sb": consts["W2sb"],
            "b1b": consts["b1b"], "b2b": consts["b2b"],
            "iota": consts["iota"], "ident": consts["ident"],
            "idx1": wrapped, "dst1": dstm, "norm1": normm,
        })

    trace = os.environ.get("KERNEL_TRACE", "0") == "1"
    res = bass_utils.run_bass_kernel_spmd(
        nc, in_maps, core_ids=list(range(NCORE)), trace=trace)
    global last_exec_time_ns
    last_exec_time_ns = res.exec_time_ns

    out = np.empty((N, OUT_C), np.float32)
    ll = np.arange(NPC)
    rows = (ll % P) * NBLK + ll // P
    for mi in range(NCORE):
        out[mi * NPC:(mi + 1) * NPC] = res.results[mi]["out"][rows]
    return out
